# revision 41
# baseline (speedup 1.0000x reference)
"""MoE layer (8 routed experts, top-2, shared experts) on 8 Trainium2 cores.

Strategy: true sparse dispatch with host-side routing. The gate (2048x8
logits + top-2 softmax) is tiny, so it runs in numpy inside kernel(); the
host gathers each expert's routed tokens (zero-padded to a fixed capacity C)
and pre-transposes them to the PE-friendly [h-partition, token] layout.
Core c then computes expert c's SwiGLU MLP over only its ~C routed tokens
(vs. all 2048 dense - a 3.6x FLOP cut). The shared expert is split 2D:
token-block (c % 4) x intermediate-half (c // 4), so each core streams only
a 1/4 slice of the shared weights and produces a partial sum the host adds
pairwise. All matmul operands are bf16 (PE runs bf16 at the same 1
cycle/row as fp32r while HBM traffic halves); accumulation stays fp32 in
PSUM. Input DMAs are issued in first-use order on the sync HWDGE ring
(FIFO), outputs drain on the scalar ring. Host does the weighted
scatter-add combine - no collectives.
"""

import sys

if "/opt/trn_rl_repo" not in sys.path:
    sys.path.insert(0, "/opt/trn_rl_repo")

import numpy as np

# ---- problem constants (hardcoded per contest contract) ----
B, S, H = 2, 1024, 2048
N = B * S                # 2048 tokens
E = 8                    # routed experts = number of cores
TOP_K = 2
M = 512                  # routed intermediate
MS = 1024                # shared intermediate total
P = 128
KT = H // P              # 16 contraction tiles
MT = M // P              # 4 routed m-tiles
MSH = 4                  # shared m-tiles per core (MS/2 = 512 wide half)
HC = H // 512            # 4 output H chunks of 512
SH_TOK = 512             # shared-expert tokens per core (token-block split)
NCORES = 8
CAP_DEFAULT = 560        # routed-token capacity per expert (seed-0 max is 554)

_CACHE = {}
_STATE = {"cap": CAP_DEFAULT}
# build-time tuning knobs (A/B-tested on hardware)
_TUNE = {
    "ps_bufs": 8,
    "first_split": 4,
    "tail_split": True,
    "loop_hints": False,
    "staggered": False,
}


def _build_program(collectives=True, loop_n=None, capacity=None):
    import concourse.bass as bass  # noqa: F401
    import concourse.mybir as mybir
    import concourse.tile as tile
    from concourse import bacc
    from contextlib import ExitStack

    C = capacity if capacity is not None else _STATE["cap"]
    f32 = mybir.dt.float32
    bf16 = mybir.dt.bfloat16
    # token chunks for the routed gate/up phase (PSUM bank is 512 f32 wide)
    chunks = []
    c0 = 0
    while c0 < C:
        cl = min(512, C - c0)
        chunks.append((c0, cl))
        c0 += cl
    nc = bacc.Bacc(None)

    xe_d = nc.declare_dram_parameter("xe", [P, KT * C], bf16, isOutput=False)
    xs_d = nc.declare_dram_parameter("xs", [P, KT * SH_TOK], bf16, isOutput=False)
    wgu_d = nc.declare_dram_parameter(
        "wgu", [MT, P, 2 * KT * P], bf16, isOutput=False
    )
    wd_d = nc.declare_dram_parameter("wd", [P, MT * H], bf16, isOutput=False)
    swgu_d = nc.declare_dram_parameter(
        "swgu", [MSH, P, 2 * KT * P], bf16, isOutput=False
    )
    swd_d = nc.declare_dram_parameter("swd", [P, MSH * H], bf16, isOutput=False)
    # routed output is h-major ([H, C]); the host transposes it back. This
    # lets the down-proj run with h as the 128-wide stationary dim (16 exact
    # tiles) and tokens as the moving dim, avoiding a mostly-padding 5th
    # token tile.
    ye_d = nc.declare_dram_parameter("ye", [H, C], bf16, isOutput=True)
    ys_d = nc.declare_dram_parameter("ys", [SH_TOK, H], bf16, isOutput=True)

    with tile.TileContext(nc) as tc:
        with (
            tc.tile_pool(name="sb", bufs=1) as sb,
            tc.tile_pool(name="sm", bufs=3) as sm,
            tc.tile_pool(name="yo", bufs=3) as ypool,
            tc.tile_pool(name="ps", bufs=_TUNE["ps_bufs"], space="PSUM") as ps,
        ):
            loop_ctx = ExitStack()
            if loop_n is not None:
                hints = mybir.ALL_ENGINES if _TUNE["loop_hints"] else ()
                loop_ctx.enter_context(
                    tc.For_i(
                        0,
                        loop_n,
                        1,
                        hint_engines=hints,
                        staggered_reset=_TUNE["staggered"],
                    )
                )

            # ---- input DMAs, issued in first-use order (sync ring is FIFO).
            # The first matmuls need only the wg half of wgu0 + the kt 0..7
            # half of xe, so those head the queue; later-phase weights follow
            # in PE consumption order.
            wgu_ts = [
                sb.tile([P, 2 * KT, P], bf16, name=f"wgu_{mt}") for mt in range(MT)
            ]
            swgu_ts = [
                sb.tile([P, 2 * KT, P], bf16, name=f"swgu_{ms}") for ms in range(MSH)
            ]

            def wgu_tile(mt):
                return wgu_ts[mt]
            xe_t = sb.tile([P, KT, C], bf16, name="xe_t")
            xs_t = sb.tile([P, KT, SH_TOK], bf16, name="xs_t")
            aT = sb.tile([P, MT, C], bf16, name="aT")
            asT = sb.tile([P, MSH, SH_TOK], bf16, name="asT")
            wd_t = sb.tile([P, MT, H], bf16, name="wd_t")
            swd_t = sb.tile([P, MSH, H], bf16, name="swd_t")

            xe_r = xe_d[:].rearrange("p (kt c) -> p kt c", c=C)
            wgu_r = [
                wgu_d[mt].rearrange("p (kt m) -> p kt m", m=P) for mt in range(MT)
            ]
            wgu0_t, wgu0_r = wgu_ts[0], wgu_r[0]
            # xe/xs ride the scalar HWDGE ring so they stream in parallel
            # with the weight queue on the sync ring.
            if _TUNE["first_split"] == 4:
                q = KT // 4
                nc.sync.dma_start(wgu0_t[:, :q], wgu0_r[:, :q])
                nc.scalar.dma_start(xe_t[:, :q], xe_r[:, :q])
                nc.sync.dma_start(wgu0_t[:, q : 2 * q], wgu0_r[:, q : 2 * q])
                nc.scalar.dma_start(xe_t[:, q : 2 * q], xe_r[:, q : 2 * q])
                nc.sync.dma_start(wgu0_t[:, 2 * q : KT], wgu0_r[:, 2 * q : KT])
                nc.scalar.dma_start(xe_t[:, 2 * q : KT], xe_r[:, 2 * q : KT])
                nc.sync.dma_start(wgu0_t[:, KT:], wgu0_r[:, KT:])
            else:
                h = KT // 2
                nc.sync.dma_start(wgu0_t[:, :KT], wgu0_r[:, :KT])
                nc.scalar.dma_start(xe_t[:, :h], xe_r[:, :h])
                nc.sync.dma_start(wgu0_t[:, KT:], wgu0_r[:, KT:])
                nc.scalar.dma_start(xe_t[:, h:], xe_r[:, h:])
            for mt in range(1, MT):
                nc.sync.dma_start(wgu_ts[mt][:], wgu_r[mt])
            nc.sync.dma_start(wd_t[:], wd_d[:].rearrange("p (mt h) -> p mt h", h=H))
            nc.scalar.dma_start(
                xs_t[:], xs_d[:].rearrange("p (kt c) -> p kt c", c=SH_TOK)
            )
            for ms in range(MSH):
                nc.sync.dma_start(
                    swgu_ts[ms][:], swgu_d[ms].rearrange("p (kt m) -> p kt m", m=P)
                )
            nc.sync.dma_start(
                swd_t[:], swd_d[:].rearrange("p (ms h) -> p ms h", h=H)
            )

            def swiglu(psG, psU, dst, cl, tag):
                sig = sm.tile([P, cl], f32, name=f"sig_{tag}", tag="sil")
                nc.scalar.activation(
                    sig[:], psG[:], mybir.ActivationFunctionType.Sigmoid
                )
                su = sm.tile([P, cl], f32, name=f"su_{tag}", tag="su")
                nc.vector.tensor_mul(su[:], sig[:], psU[:])
                nc.vector.tensor_mul(dst, su[:], psG[:])

            # ---------- Phase R-GU: routed expert gate/up + SwiGLU ----------
            for mt in range(MT):
                for c0, cl in chunks:
                    psG = ps.tile([P, cl], f32, name=f"psG_{mt}_{c0}", tag="ps")
                    for kt in range(KT):
                        nc.tensor.matmul(
                            psG[:],
                            wgu_tile(mt)[:, kt, :],
                            xe_t[:, kt, c0 : c0 + cl],
                            start=(kt == 0),
                            stop=(kt == KT - 1),
                        )
                    psU = ps.tile([P, cl], f32, name=f"psU_{mt}_{c0}", tag="ps")
                    for kt in range(KT):
                        nc.tensor.matmul(
                            psU[:],
                            wgu_tile(mt)[:, KT + kt, :],
                            xe_t[:, kt, c0 : c0 + cl],
                            start=(kt == 0),
                            stop=(kt == KT - 1),
                        )
                    swiglu(psG, psU, aT[:, mt, c0 : c0 + cl], cl, f"r{mt}_{c0}")

            # ---------- Phase R-D: routed down projection (h-major) ----------
            # out[h, t] = sum_m wd[m, h] * a[m, t]: h is the stationary dim
            # (16 exact 128-tiles), tokens are the moving dim (C-chunked).
            for ht in range(KT):
                h0 = ht * P
                yo = ypool.tile([P, C], bf16, name=f"yo_{ht}", tag="yoh")
                for ci, (c0, cl) in enumerate(chunks):
                    psY = ps.tile([P, cl], f32, name=f"psY_{ht}_{c0}", tag="ps")
                    for mt in range(MT):
                        nc.tensor.matmul(
                            psY[:],
                            wd_t[:, mt, h0 : h0 + P],
                            aT[:, mt, c0 : c0 + cl],
                            start=(mt == 0),
                            stop=(mt == MT - 1),
                        )
                    if (ht + ci) % 2 == 0:
                        nc.scalar.copy(yo[:, c0 : c0 + cl], psY[:])
                    else:
                        nc.vector.tensor_copy(yo[:, c0 : c0 + cl], psY[:])
                nc.scalar.dma_start(ye_d[h0 : h0 + P, :], yo[:])

            # ---------- Phase S-GU: shared expert gate/up + SwiGLU ----------
            for ms in range(MSH):
                psG = ps.tile([P, SH_TOK], f32, name=f"psGs_{ms}", tag="ps")
                for kt in range(KT):
                    nc.tensor.matmul(
                        psG[:],
                        swgu_ts[ms][:, kt, :],
                        xs_t[:, kt, :],
                        start=(kt == 0),
                        stop=(kt == KT - 1),
                    )
                psU = ps.tile([P, SH_TOK], f32, name=f"psUs_{ms}", tag="ps")
                for kt in range(KT):
                    nc.tensor.matmul(
                        psU[:],
                        swgu_ts[ms][:, KT + kt, :],
                        xs_t[:, kt, :],
                        start=(kt == 0),
                        stop=(kt == KT - 1),
                    )
                swiglu(psG, psU, asT[:, ms, :], SH_TOK, f"s{ms}")

            # ---------- Phase S-D: shared down projection (partial sums) ----
            last_ti = SH_TOK // P - 1
            for ti in range(SH_TOK // P):
                t0 = ti * P
                yo = ypool.tile([P, H], bf16, name=f"yos_{ti}", tag="yo")
                for hc in range(HC):
                    h0 = hc * 512
                    psY = ps.tile([P, 512], f32, name=f"psYs_{ti}_{hc}", tag="ps")
                    for ms in range(MSH):
                        nc.tensor.matmul(
                            psY[:],
                            asT[:, ms, t0 : t0 + P],
                            swd_t[:, ms, h0 : h0 + 512],
                            start=(ms == 0),
                            stop=(ms == MSH - 1),
                        )
                    if ti == last_ti and _TUNE["tail_split"]:
                        # drain the final tile per-hc so the tail DMA is small
                        if hc % 2 == 0:
                            nc.scalar.copy(yo[:, h0 : h0 + 512], psY[:])
                        else:
                            nc.vector.tensor_copy(yo[:, h0 : h0 + 512], psY[:])
                        nc.scalar.dma_start(
                            ys_d[t0 : t0 + P, h0 : h0 + 512],
                            yo[:, h0 : h0 + 512],
                        )
                    elif hc % 2 == 0:
                        nc.scalar.copy(yo[:, h0 : h0 + 512], psY[:])
                    else:
                        nc.vector.tensor_copy(yo[:, h0 : h0 + 512], psY[:])
                        nc.scalar.dma_start(
                            ys_d[t0 : t0 + P, h0 - 512 : h0 + 512],
                            yo[:, h0 - 512 : h0 + 512],
                        )

            loop_ctx.close()

    nc.finalize()
    return nc


def _route(x, gate_w):
    """Exact top-2 routing in fp32 numpy. Returns per-expert token lists and
    combine weights."""
    logits = x @ gate_w.T                              # [N, E]
    # top-2 (descending, ties -> lower index, matching jax.lax.top_k)
    i1 = np.argmax(logits, axis=1)
    l1 = logits[np.arange(N), i1]
    masked = logits.copy()
    masked[np.arange(N), i1] = -np.inf
    i2 = np.argmax(masked, axis=1)
    l2 = masked[np.arange(N), i2]
    # softmax over the two selected logits
    ew = np.exp(l2 - l1)
    w1 = 1.0 / (1.0 + ew)
    w2 = ew * w1
    toks, wts = [], []
    for e in range(E):
        m1 = i1 == e
        m2 = i2 == e
        t = np.concatenate([np.nonzero(m1)[0], np.nonzero(m2)[0]])
        w = np.concatenate([w1[m1], w2[m2]]).astype(np.float32)
        toks.append(t)
        wts.append(w)
    return toks, wts


def _prep_in_maps(inputs) -> list:
    import ml_dtypes

    bf16 = ml_dtypes.bfloat16
    x = np.ascontiguousarray(
        np.asarray(inputs["hidden_states"], dtype=np.float32).reshape(N, H)
    )
    gate_w = np.asarray(inputs["gate_w"], dtype=np.float32)
    Wg = np.asarray(inputs["Wg"], dtype=np.float32)
    Wu = np.asarray(inputs["Wu"], dtype=np.float32)
    Wd = np.asarray(inputs["Wd"], dtype=np.float32)
    sWg = np.asarray(inputs["sWg"], dtype=np.float32)
    sWu = np.asarray(inputs["sWu"], dtype=np.float32)
    sWd = np.asarray(inputs["sWd"], dtype=np.float32)

    toks, wts = _route(x, gate_w)
    need = max(len(t) for t in toks)
    cap = max(CAP_DEFAULT, -(-need // 8) * 8)
    _STATE["cap"] = cap
    _STATE["toks"] = toks
    _STATE["wts"] = wts
    C = cap

    def up_tiles(w, mtiles):  # [H, Mw] -> [mtiles, P, KT*P] partition-major
        return np.ascontiguousarray(
            w.reshape(KT, P, mtiles, P).transpose(2, 1, 0, 3).reshape(mtiles, P, KT * P)
        ).astype(bf16)

    def down_tiles(w, mtiles):  # [Mw, H] -> [P, mtiles*H]
        return np.ascontiguousarray(
            w.reshape(mtiles, P, H).transpose(1, 0, 2).reshape(P, mtiles * H)
        ).astype(bf16)

    def xT(xg, width):  # [n<=width, H] -> [P, KT*width] transposed + padded
        n = xg.shape[0]
        out = np.zeros((P, KT, width), dtype=bf16)
        out[:, :, :n] = xg.reshape(n, KT, P).transpose(2, 1, 0).astype(bf16)
        return np.ascontiguousarray(out.reshape(P, KT * width))

    # shared-expert 2D split: intermediate half by c // 4, token block by c % 4
    swgu_half = []
    swd_half = []
    for mh in range(2):
        sg = up_tiles(sWg[:, mh * 512 : (mh + 1) * 512], MSH)
        su = up_tiles(sWu[:, mh * 512 : (mh + 1) * 512], MSH)
        swgu_half.append(np.ascontiguousarray(np.concatenate([sg, su], axis=2)))
        swd_half.append(down_tiles(sWd[mh * 512 : (mh + 1) * 512, :], MSH))
    xs_block = [
        xT(x[tb * SH_TOK : (tb + 1) * SH_TOK], SH_TOK) for tb in range(4)
    ]

    in_maps = []
    for c in range(NCORES):
        wg_t = up_tiles(Wg[c], MT)
        wu_t = up_tiles(Wu[c], MT)
        in_maps.append(
            {
                "xe": xT(x[toks[c]], C),
                "xs": xs_block[c % 4],
                "wgu": np.ascontiguousarray(np.concatenate([wg_t, wu_t], axis=2)),
                "wd": down_tiles(Wd[c], MT),
                "swgu": swgu_half[c // 4],
                "swd": swd_half[c // 4],
            }
        )
    return in_maps


def _unshard(results) -> np.ndarray:
    toks, wts = _STATE["toks"], _STATE["wts"]
    y = np.empty((N, H), dtype=np.float32)
    for tb in range(4):
        y[tb * SH_TOK : (tb + 1) * SH_TOK] = results[tb]["ys"].astype(
            np.float32
        ) + results[tb + 4]["ys"].astype(np.float32)
    for e in range(NCORES):
        t = toks[e]
        ye = results[e]["ye"].astype(np.float32).T  # [H, C] -> [C, H]
        y[t] += wts[e][:, None] * ye[: len(t)]
    return y.reshape(B, S, H)


def kernel(**inputs) -> np.ndarray:
    from concourse.bass_utils import run_bass_kernel_spmd

    in_maps = _prep_in_maps(inputs)

    key = ("nc", _STATE["cap"])
    if key not in _CACHE:
        _CACHE[key] = _build_program(capacity=_STATE["cap"])
    nc = _CACHE[key]

    res = run_bass_kernel_spmd(nc, in_maps, list(range(NCORES))).results
    return _unshard(res)


if __name__ == "__main__":
    # smoke test against the local reference
    sys.path.insert(0, "/root/problem")
    import reference

    inp = reference.setup_inputs()
    expected = np.asarray(reference.reference(**inp))
    actual = kernel(**{k: np.asarray(v) for k, v in inp.items()})
    err = np.linalg.norm(actual - expected) / np.linalg.norm(expected)
    print("Relative error:", err)


# revision 43
# speedup vs baseline: 1.0253x; 1.0253x over previous
"""MoE layer (8 routed experts, top-2, shared experts) on 8 Trainium2 cores.

Strategy: true sparse dispatch with host-side routing. The gate (2048x8
logits + top-2 softmax) is tiny, so it runs in numpy inside kernel(); the
host gathers each expert's routed tokens (zero-padded to a fixed capacity C)
and pre-transposes them to the PE-friendly [h-partition, token] layout.
Core c then computes expert c's SwiGLU MLP over only its ~C routed tokens
(vs. all 2048 dense - a 3.6x FLOP cut). The shared expert is split 2D:
token-block (c % 4) x intermediate-half (c // 4), so each core streams only
a 1/4 slice of the shared weights and produces a partial sum the host adds
pairwise. All matmul operands are bf16 (PE runs bf16 at the same 1
cycle/row as fp32r while HBM traffic halves); accumulation stays fp32 in
PSUM. Input DMAs are issued in first-use order on the sync HWDGE ring
(FIFO), outputs drain on the scalar ring. Host does the weighted
scatter-add combine - no collectives.
"""

import sys

if "/opt/trn_rl_repo" not in sys.path:
    sys.path.insert(0, "/opt/trn_rl_repo")

import numpy as np

# ---- problem constants (hardcoded per contest contract) ----
B, S, H = 2, 1024, 2048
N = B * S                # 2048 tokens
E = 8                    # routed experts = number of cores
TOP_K = 2
M = 512                  # routed intermediate
MS = 1024                # shared intermediate total
P = 128
KT = H // P              # 16 contraction tiles
MT = M // P              # 4 routed m-tiles
MSH = 4                  # shared m-tiles per core (MS/2 = 512 wide half)
HC = H // 512            # 4 output H chunks of 512
SH_TOK = 512             # shared-expert tokens per core (token-block split)
NCORES = 8
CAP_DEFAULT = 560        # routed-token capacity per expert (seed-0 max is 554)

_CACHE = {}
_STATE = {"cap": CAP_DEFAULT}
# build-time tuning knobs (A/B-tested on hardware)
_TUNE = {
    "ps_bufs": 8,
    "first_split": 4,
    "tail_split": True,
    "loop_hints": False,
    "staggered": False,
    "warmup_n": 28,
}


def _build_program(collectives=True, loop_n=None, capacity=None):
    import concourse.bass as bass  # noqa: F401
    import concourse.mybir as mybir
    import concourse.tile as tile
    from concourse import bacc
    from contextlib import ExitStack

    C = capacity if capacity is not None else _STATE["cap"]
    f32 = mybir.dt.float32
    bf16 = mybir.dt.bfloat16
    # token chunks for the routed gate/up phase (PSUM bank is 512 f32 wide)
    chunks = []
    c0 = 0
    while c0 < C:
        cl = min(512, C - c0)
        chunks.append((c0, cl))
        c0 += cl
    nc = bacc.Bacc(None)

    xe_d = nc.declare_dram_parameter("xe", [P, KT * C], bf16, isOutput=False)
    xs_d = nc.declare_dram_parameter("xs", [P, KT * SH_TOK], bf16, isOutput=False)
    wgu_d = nc.declare_dram_parameter(
        "wgu", [MT, P, 2 * KT * P], bf16, isOutput=False
    )
    wd_d = nc.declare_dram_parameter("wd", [P, MT * H], bf16, isOutput=False)
    swgu_d = nc.declare_dram_parameter(
        "swgu", [MSH, P, 2 * KT * P], bf16, isOutput=False
    )
    swd_d = nc.declare_dram_parameter("swd", [P, MSH * H], bf16, isOutput=False)
    # routed output is h-major ([H, C]); the host transposes it back. This
    # lets the down-proj run with h as the 128-wide stationary dim (16 exact
    # tiles) and tokens as the moving dim, avoiding a mostly-padding 5th
    # token tile.
    ye_d = nc.declare_dram_parameter("ye", [H, C], bf16, isOutput=True)
    ys_d = nc.declare_dram_parameter("ys", [SH_TOK, H], bf16, isOutput=True)

    with tile.TileContext(nc) as tc:
        with (
            tc.tile_pool(name="sb", bufs=1) as sb,
            tc.tile_pool(name="sm", bufs=3) as sm,
            tc.tile_pool(name="yo", bufs=3) as ypool,
            tc.tile_pool(name="ps", bufs=_TUNE["ps_bufs"], space="PSUM") as ps,
        ):
            loop_ctx = ExitStack()
            if loop_n is not None:
                hints = mybir.ALL_ENGINES if _TUNE["loop_hints"] else ()
                loop_ctx.enter_context(
                    tc.For_i(
                        0,
                        loop_n,
                        1,
                        hint_engines=hints,
                        staggered_reset=_TUNE["staggered"],
                    )
                )

            # ---- PE warm-up: the tensor engine runs at half clock for its
            # first ~3us of continuous execution after an idle gap, and the
            # iteration starts with a ~5us DMA wait. Dependency-free filler
            # matmuls (into never-read PSUM scratch) bridge that window so
            # the real matmuls start at full clock.
            if _TUNE["warmup_n"]:
                warm = sb.tile([P, 256], bf16, name="warm")
                nc.vector.memset(warm[:], 0.0)
                for wi in range(_TUNE["warmup_n"]):
                    pw = ps.tile([P, 256], f32, name=f"pwarm_{wi}", tag="ps")
                    nc.tensor.matmul(
                        pw[:], warm[:, :P], warm[:], start=True, stop=True
                    )

            # ---- input DMAs, issued in first-use order (sync ring is FIFO).
            # The first matmuls need only the wg half of wgu0 + the kt 0..7
            # half of xe, so those head the queue; later-phase weights follow
            # in PE consumption order.
            wgu_ts = [
                sb.tile([P, 2 * KT, P], bf16, name=f"wgu_{mt}") for mt in range(MT)
            ]
            swgu_ts = [
                sb.tile([P, 2 * KT, P], bf16, name=f"swgu_{ms}") for ms in range(MSH)
            ]

            def wgu_tile(mt):
                return wgu_ts[mt]
            xe_t = sb.tile([P, KT, C], bf16, name="xe_t")
            xs_t = sb.tile([P, KT, SH_TOK], bf16, name="xs_t")
            aT = sb.tile([P, MT, C], bf16, name="aT")
            asT = sb.tile([P, MSH, SH_TOK], bf16, name="asT")
            wd_t = sb.tile([P, MT, H], bf16, name="wd_t")
            swd_t = sb.tile([P, MSH, H], bf16, name="swd_t")

            xe_r = xe_d[:].rearrange("p (kt c) -> p kt c", c=C)
            wgu_r = [
                wgu_d[mt].rearrange("p (kt m) -> p kt m", m=P) for mt in range(MT)
            ]
            wgu0_t, wgu0_r = wgu_ts[0], wgu_r[0]
            # xe/xs ride the scalar HWDGE ring so they stream in parallel
            # with the weight queue on the sync ring.
            if _TUNE["first_split"] == 4:
                q = KT // 4
                nc.sync.dma_start(wgu0_t[:, :q], wgu0_r[:, :q])
                nc.scalar.dma_start(xe_t[:, :q], xe_r[:, :q])
                nc.sync.dma_start(wgu0_t[:, q : 2 * q], wgu0_r[:, q : 2 * q])
                nc.scalar.dma_start(xe_t[:, q : 2 * q], xe_r[:, q : 2 * q])
                nc.sync.dma_start(wgu0_t[:, 2 * q : KT], wgu0_r[:, 2 * q : KT])
                nc.scalar.dma_start(xe_t[:, 2 * q : KT], xe_r[:, 2 * q : KT])
                nc.sync.dma_start(wgu0_t[:, KT:], wgu0_r[:, KT:])
            else:
                h = KT // 2
                nc.sync.dma_start(wgu0_t[:, :KT], wgu0_r[:, :KT])
                nc.scalar.dma_start(xe_t[:, :h], xe_r[:, :h])
                nc.sync.dma_start(wgu0_t[:, KT:], wgu0_r[:, KT:])
                nc.scalar.dma_start(xe_t[:, h:], xe_r[:, h:])
            for mt in range(1, MT):
                nc.sync.dma_start(wgu_ts[mt][:], wgu_r[mt])
            nc.sync.dma_start(wd_t[:], wd_d[:].rearrange("p (mt h) -> p mt h", h=H))
            nc.scalar.dma_start(
                xs_t[:], xs_d[:].rearrange("p (kt c) -> p kt c", c=SH_TOK)
            )
            for ms in range(MSH):
                nc.sync.dma_start(
                    swgu_ts[ms][:], swgu_d[ms].rearrange("p (kt m) -> p kt m", m=P)
                )
            nc.sync.dma_start(
                swd_t[:], swd_d[:].rearrange("p (ms h) -> p ms h", h=H)
            )

            def swiglu(psG, psU, dst, cl, tag):
                sig = sm.tile([P, cl], f32, name=f"sig_{tag}", tag="sil")
                nc.scalar.activation(
                    sig[:], psG[:], mybir.ActivationFunctionType.Sigmoid
                )
                su = sm.tile([P, cl], f32, name=f"su_{tag}", tag="su")
                nc.vector.tensor_mul(su[:], sig[:], psU[:])
                nc.vector.tensor_mul(dst, su[:], psG[:])

            # ---------- Phase R-GU: routed expert gate/up + SwiGLU ----------
            for mt in range(MT):
                for c0, cl in chunks:
                    psG = ps.tile([P, cl], f32, name=f"psG_{mt}_{c0}", tag="ps")
                    for kt in range(KT):
                        nc.tensor.matmul(
                            psG[:],
                            wgu_tile(mt)[:, kt, :],
                            xe_t[:, kt, c0 : c0 + cl],
                            start=(kt == 0),
                            stop=(kt == KT - 1),
                        )
                    psU = ps.tile([P, cl], f32, name=f"psU_{mt}_{c0}", tag="ps")
                    for kt in range(KT):
                        nc.tensor.matmul(
                            psU[:],
                            wgu_tile(mt)[:, KT + kt, :],
                            xe_t[:, kt, c0 : c0 + cl],
                            start=(kt == 0),
                            stop=(kt == KT - 1),
                        )
                    swiglu(psG, psU, aT[:, mt, c0 : c0 + cl], cl, f"r{mt}_{c0}")

            # ---------- Phase R-D: routed down projection (h-major) ----------
            # out[h, t] = sum_m wd[m, h] * a[m, t]: h is the stationary dim
            # (16 exact 128-tiles), tokens are the moving dim (C-chunked).
            for ht in range(KT):
                h0 = ht * P
                yo = ypool.tile([P, C], bf16, name=f"yo_{ht}", tag="yoh")
                for ci, (c0, cl) in enumerate(chunks):
                    psY = ps.tile([P, cl], f32, name=f"psY_{ht}_{c0}", tag="ps")
                    for mt in range(MT):
                        nc.tensor.matmul(
                            psY[:],
                            wd_t[:, mt, h0 : h0 + P],
                            aT[:, mt, c0 : c0 + cl],
                            start=(mt == 0),
                            stop=(mt == MT - 1),
                        )
                    if (ht + ci) % 2 == 0:
                        nc.scalar.copy(yo[:, c0 : c0 + cl], psY[:])
                    else:
                        nc.vector.tensor_copy(yo[:, c0 : c0 + cl], psY[:])
                nc.scalar.dma_start(ye_d[h0 : h0 + P, :], yo[:])

            # ---------- Phase S-GU: shared expert gate/up + SwiGLU ----------
            for ms in range(MSH):
                psG = ps.tile([P, SH_TOK], f32, name=f"psGs_{ms}", tag="ps")
                for kt in range(KT):
                    nc.tensor.matmul(
                        psG[:],
                        swgu_ts[ms][:, kt, :],
                        xs_t[:, kt, :],
                        start=(kt == 0),
                        stop=(kt == KT - 1),
                    )
                psU = ps.tile([P, SH_TOK], f32, name=f"psUs_{ms}", tag="ps")
                for kt in range(KT):
                    nc.tensor.matmul(
                        psU[:],
                        swgu_ts[ms][:, KT + kt, :],
                        xs_t[:, kt, :],
                        start=(kt == 0),
                        stop=(kt == KT - 1),
                    )
                swiglu(psG, psU, asT[:, ms, :], SH_TOK, f"s{ms}")

            # ---------- Phase S-D: shared down projection (partial sums) ----
            last_ti = SH_TOK // P - 1
            for ti in range(SH_TOK // P):
                t0 = ti * P
                yo = ypool.tile([P, H], bf16, name=f"yos_{ti}", tag="yo")
                for hc in range(HC):
                    h0 = hc * 512
                    psY = ps.tile([P, 512], f32, name=f"psYs_{ti}_{hc}", tag="ps")
                    for ms in range(MSH):
                        nc.tensor.matmul(
                            psY[:],
                            asT[:, ms, t0 : t0 + P],
                            swd_t[:, ms, h0 : h0 + 512],
                            start=(ms == 0),
                            stop=(ms == MSH - 1),
                        )
                    if ti == last_ti and _TUNE["tail_split"]:
                        # drain the final tile per-hc so the tail DMA is small
                        if hc % 2 == 0:
                            nc.scalar.copy(yo[:, h0 : h0 + 512], psY[:])
                        else:
                            nc.vector.tensor_copy(yo[:, h0 : h0 + 512], psY[:])
                        nc.scalar.dma_start(
                            ys_d[t0 : t0 + P, h0 : h0 + 512],
                            yo[:, h0 : h0 + 512],
                        )
                    elif hc % 2 == 0:
                        nc.scalar.copy(yo[:, h0 : h0 + 512], psY[:])
                    else:
                        nc.vector.tensor_copy(yo[:, h0 : h0 + 512], psY[:])
                        nc.scalar.dma_start(
                            ys_d[t0 : t0 + P, h0 - 512 : h0 + 512],
                            yo[:, h0 - 512 : h0 + 512],
                        )

            loop_ctx.close()

    nc.finalize()
    return nc


def _route(x, gate_w):
    """Exact top-2 routing in fp32 numpy. Returns per-expert token lists and
    combine weights."""
    logits = x @ gate_w.T                              # [N, E]
    # top-2 (descending, ties -> lower index, matching jax.lax.top_k)
    i1 = np.argmax(logits, axis=1)
    l1 = logits[np.arange(N), i1]
    masked = logits.copy()
    masked[np.arange(N), i1] = -np.inf
    i2 = np.argmax(masked, axis=1)
    l2 = masked[np.arange(N), i2]
    # softmax over the two selected logits
    ew = np.exp(l2 - l1)
    w1 = 1.0 / (1.0 + ew)
    w2 = ew * w1
    toks, wts = [], []
    for e in range(E):
        m1 = i1 == e
        m2 = i2 == e
        t = np.concatenate([np.nonzero(m1)[0], np.nonzero(m2)[0]])
        w = np.concatenate([w1[m1], w2[m2]]).astype(np.float32)
        toks.append(t)
        wts.append(w)
    return toks, wts


def _prep_in_maps(inputs) -> list:
    import ml_dtypes

    bf16 = ml_dtypes.bfloat16
    x = np.ascontiguousarray(
        np.asarray(inputs["hidden_states"], dtype=np.float32).reshape(N, H)
    )
    gate_w = np.asarray(inputs["gate_w"], dtype=np.float32)
    Wg = np.asarray(inputs["Wg"], dtype=np.float32)
    Wu = np.asarray(inputs["Wu"], dtype=np.float32)
    Wd = np.asarray(inputs["Wd"], dtype=np.float32)
    sWg = np.asarray(inputs["sWg"], dtype=np.float32)
    sWu = np.asarray(inputs["sWu"], dtype=np.float32)
    sWd = np.asarray(inputs["sWd"], dtype=np.float32)

    toks, wts = _route(x, gate_w)
    need = max(len(t) for t in toks)
    cap = max(CAP_DEFAULT, -(-need // 8) * 8)
    _STATE["cap"] = cap
    _STATE["toks"] = toks
    _STATE["wts"] = wts
    C = cap

    def up_tiles(w, mtiles):  # [H, Mw] -> [mtiles, P, KT*P] partition-major
        return np.ascontiguousarray(
            w.reshape(KT, P, mtiles, P).transpose(2, 1, 0, 3).reshape(mtiles, P, KT * P)
        ).astype(bf16)

    def down_tiles(w, mtiles):  # [Mw, H] -> [P, mtiles*H]
        return np.ascontiguousarray(
            w.reshape(mtiles, P, H).transpose(1, 0, 2).reshape(P, mtiles * H)
        ).astype(bf16)

    def xT(xg, width):  # [n<=width, H] -> [P, KT*width] transposed + padded
        n = xg.shape[0]
        out = np.zeros((P, KT, width), dtype=bf16)
        out[:, :, :n] = xg.reshape(n, KT, P).transpose(2, 1, 0).astype(bf16)
        return np.ascontiguousarray(out.reshape(P, KT * width))

    # shared-expert 2D split: intermediate half by c // 4, token block by c % 4
    swgu_half = []
    swd_half = []
    for mh in range(2):
        sg = up_tiles(sWg[:, mh * 512 : (mh + 1) * 512], MSH)
        su = up_tiles(sWu[:, mh * 512 : (mh + 1) * 512], MSH)
        swgu_half.append(np.ascontiguousarray(np.concatenate([sg, su], axis=2)))
        swd_half.append(down_tiles(sWd[mh * 512 : (mh + 1) * 512, :], MSH))
    xs_block = [
        xT(x[tb * SH_TOK : (tb + 1) * SH_TOK], SH_TOK) for tb in range(4)
    ]

    in_maps = []
    for c in range(NCORES):
        wg_t = up_tiles(Wg[c], MT)
        wu_t = up_tiles(Wu[c], MT)
        in_maps.append(
            {
                "xe": xT(x[toks[c]], C),
                "xs": xs_block[c % 4],
                "wgu": np.ascontiguousarray(np.concatenate([wg_t, wu_t], axis=2)),
                "wd": down_tiles(Wd[c], MT),
                "swgu": swgu_half[c // 4],
                "swd": swd_half[c // 4],
            }
        )
    return in_maps


def _unshard(results) -> np.ndarray:
    toks, wts = _STATE["toks"], _STATE["wts"]
    y = np.empty((N, H), dtype=np.float32)
    for tb in range(4):
        y[tb * SH_TOK : (tb + 1) * SH_TOK] = results[tb]["ys"].astype(
            np.float32
        ) + results[tb + 4]["ys"].astype(np.float32)
    for e in range(NCORES):
        t = toks[e]
        ye = results[e]["ye"].astype(np.float32).T  # [H, C] -> [C, H]
        y[t] += wts[e][:, None] * ye[: len(t)]
    return y.reshape(B, S, H)


def kernel(**inputs) -> np.ndarray:
    from concourse.bass_utils import run_bass_kernel_spmd

    in_maps = _prep_in_maps(inputs)

    key = ("nc", _STATE["cap"])
    if key not in _CACHE:
        _CACHE[key] = _build_program(capacity=_STATE["cap"])
    nc = _CACHE[key]

    res = run_bass_kernel_spmd(nc, in_maps, list(range(NCORES))).results
    return _unshard(res)


if __name__ == "__main__":
    # smoke test against the local reference
    sys.path.insert(0, "/root/problem")
    import reference

    inp = reference.setup_inputs()
    expected = np.asarray(reference.reference(**inp))
    actual = kernel(**{k: np.asarray(v) for k, v in inp.items()})
    err = np.linalg.norm(actual - expected) / np.linalg.norm(expected)
    print("Relative error:", err)
